# revision 7
# baseline (speedup 1.0000x reference)
"""BinaryLinear kernel for 8x Trainium2 NeuronCores.

Computes out = sign(x) @ sign(weight).T + bias for
x [8192, 4096] f32, weight [4096, 4096] f32, bias [4096] f32.

Sharding: 4 token-groups x 2 out-column-groups grid (core c = r*2 + h):
  core reads x rows [r*2048:(r+1)*2048] and weight rows [h*2048:(h+1)*2048],
  writes out block [r, h] of shape [2048, 2048].
This minimizes per-core HBM traffic (x/R + w/C + out/8).

Per-core pipeline:
  Phase A: load f32 tiles, binarize with ACT sign -> bf16, stage to DRAM.
  Phase B: dma_start_transpose (xbar) the bf16 staging back as [k, *] tiles,
           convert to fp8 (+-1 exact), run fp8 matmuls accumulating in PSUM
           (optionally DoubleRow), add bias on eviction, DMA out.
"""

import os
import sys
import time

sys.path.insert(0, "/opt/trn_rl_repo")

# The kernel executes through the axon PJRT backend; a JAX_PLATFORMS=cpu pin
# (common for reference-side jax) would hide the NeuronCores. Clear it if jax
# has not been imported yet.
if "jax" not in sys.modules and os.environ.get("JAX_PLATFORMS") in ("cpu",):
    del os.environ["JAX_PLATFORMS"]

import numpy as np

import concourse.bass as bass
import concourse.mybir as mybir
import concourse.tile as tile

N_TOK = 8192
IN_F = 4096
OUT_F = 4096
R = 4  # token groups
C = 2  # out-column groups
N_CORES = 8
TOK_SH = N_TOK // R  # 2048 tokens per core
OUT_SH = OUT_F // C  # 2048 out columns per core
P = 128
KS = IN_F // P  # 32 k-subtiles
NQ = 4  # token quarters within a core
TOKQ = TOK_SH // NQ  # 512
O_CHUNK = 512
N_OCH = OUT_SH // O_CHUNK  # 4

USE_DOUBLE_ROW = True

f32 = mybir.dt.float32
bf16 = mybir.dt.bfloat16
fp8 = mybir.dt.float8e4


def _split_multi_waits(nc, limit=1):
    """Split instructions carrying >limit semaphore waits.

    The walrus build here rejects instructions with more than one sync-wait
    command ("Too many sync wait commands"), while Tile freely attaches
    several. Engines execute their streams in order, so excess waits can be
    moved onto NoOp instructions inserted immediately before the original.
    """
    for f in nc.m.functions:
        for bb in f.blocks:
            new = []
            for inst in bb.instructions:
                si = inst.sync_info
                if si is not None and len(si.on_wait) > limit:
                    waits = list(si.on_wait)
                    extra, keep = waits[:-limit], waits[-limit:]
                    for j, w in enumerate(extra):
                        new.append(
                            mybir.InstNoOp(
                                name=f"{inst.name}-w{j}",
                                engine=inst.engine,
                                sync_info=mybir.SyncInfo(on_wait=[w], on_update=[]),
                            )
                        )
                    inst.sync_info = mybir.SyncInfo(
                        on_wait=keep, on_update=list(si.on_update)
                    )
                new.append(inst)
            bb.instructions = new


def build_nc():
    nc = bass.Bass()
    x = nc.declare_dram_parameter("x", [TOK_SH, IN_F], f32, isOutput=False)
    w = nc.declare_dram_parameter("w", [OUT_SH, IN_F], f32, isOutput=False)
    b = nc.declare_dram_parameter("b", [P, OUT_SH], f32, isOutput=False)
    y = nc.declare_dram_parameter("y", [TOK_SH, OUT_SH], f32, isOutput=True)

    with tile.TileContext(nc) as tc:
        with (
            tc.tile_pool(name="dram", bufs=1, space="DRAM") as dram,
            tc.tile_pool(name="const", bufs=1) as const,
            tc.tile_pool(name="a_in", bufs=3) as a_in,
            tc.tile_pool(name="a_bf", bufs=3) as a_bf,
            tc.tile_pool(name="wt_stage", bufs=2) as wt_stage,
            tc.tile_pool(name="xt_stage", bufs=3) as xt_stage,
            tc.tile_pool(name="wbt", bufs=1) as wbt_pool,
            tc.tile_pool(name="xbt", bufs=2) as xbt_pool,
            tc.tile_pool(name="psum", bufs=4, space="PSUM") as psum_pool,
            tc.tile_pool(name="outp", bufs=3) as out_pool,
        ):
            # bias, already broadcast to [128, OUT_SH] on the host
            bias_bc = const.tile([P, OUT_SH], f32)
            nc.sync.dma_start(out=bias_bc[:], in_=b[:])

            # ---- Phase A (w): binarize weight shard to bf16 in DRAM ----
            wb_dram = dram.tile([OUT_SH, IN_F], bf16, tag="wb_dram")
            for rt in range(OUT_SH // P):
                for kh in range(2):
                    a = a_in.tile([P, IN_F // 2], f32, tag="a_in")
                    nc.sync.dma_start(
                        out=a[:],
                        in_=w[rt * P : (rt + 1) * P, kh * (IN_F // 2) : (kh + 1) * (IN_F // 2)],
                    )
                    s = a_bf.tile([P, IN_F // 2], bf16, tag="a_bf")
                    nc.scalar.sign(s[:], a[:])
                    nc.sync.dma_start(
                        out=wb_dram[rt * P : (rt + 1) * P, kh * (IN_F // 2) : (kh + 1) * (IN_F // 2)],
                        in_=s[:],
                    )

            # ---- wbT: transpose-load + fp8 convert; fully resident ----
            wbt = wbt_pool.tile([P, KS, OUT_SH], fp8)
            for ks in range(KS):
                st = wt_stage.tile([P, OUT_SH], bf16, tag="wt_stage")
                nc.sync.dma_start_transpose(st[:], wb_dram[:, ks * P : (ks + 1) * P])
                nc.vector.tensor_copy(wbt[:, ks, :], st[:])

            # ---- per token-quarter: stage x, transpose-load, matmul ----
            for q in range(NQ):
                xb_dram = dram.tile([TOKQ, IN_F], bf16, tag=f"xb_dram_{q}")
                for rt in range(TOKQ // P):
                    row0 = q * TOKQ + rt * P
                    for kh in range(2):
                        a = a_in.tile([P, IN_F // 2], f32, tag="a_in")
                        nc.sync.dma_start(
                            out=a[:],
                            in_=x[row0 : row0 + P, kh * (IN_F // 2) : (kh + 1) * (IN_F // 2)],
                        )
                        s = a_bf.tile([P, IN_F // 2], bf16, tag="a_bf")
                        nc.scalar.sign(s[:], a[:])
                        nc.sync.dma_start(
                            out=xb_dram[rt * P : (rt + 1) * P, kh * (IN_F // 2) : (kh + 1) * (IN_F // 2)],
                            in_=s[:],
                        )

                xbt = xbt_pool.tile([P, KS, TOKQ], fp8, tag="xbt")
                for ks in range(KS):
                    st = xt_stage.tile([P, TOKQ], bf16, tag="xt_stage")
                    nc.sync.dma_start_transpose(st[:], xb_dram[:, ks * P : (ks + 1) * P])
                    nc.vector.tensor_copy(xbt[:, ks, :], st[:])

                for nt in range(TOKQ // P):
                    out_sb = out_pool.tile([P, OUT_SH], f32, tag="out_sb")
                    for oc in range(N_OCH):
                        ps = psum_pool.tile([P, O_CHUNK], f32, tag="ps")
                        if USE_DOUBLE_ROW:
                            for kp in range(KS // 2):
                                nc.tensor.matmul(
                                    ps[:],
                                    lhsT=xbt[:, 2 * kp : 2 * kp + 2, nt * P : (nt + 1) * P],
                                    rhs=wbt[:, 2 * kp : 2 * kp + 2, oc * O_CHUNK : (oc + 1) * O_CHUNK],
                                    start=(kp == 0),
                                    stop=(kp == KS // 2 - 1),
                                    perf_mode=mybir.MatmulPerfMode.DoubleRow,
                                )
                        else:
                            for ks in range(KS):
                                nc.tensor.matmul(
                                    ps[:],
                                    lhsT=xbt[:, ks, nt * P : (nt + 1) * P],
                                    rhs=wbt[:, ks, oc * O_CHUNK : (oc + 1) * O_CHUNK],
                                    start=(ks == 0),
                                    stop=(ks == KS - 1),
                                )
                        nc.vector.tensor_add(
                            out=out_sb[:, oc * O_CHUNK : (oc + 1) * O_CHUNK],
                            in0=ps[:],
                            in1=bias_bc[:, oc * O_CHUNK : (oc + 1) * O_CHUNK],
                        )
                    row0 = q * TOKQ + nt * P
                    nc.sync.dma_start(out=y[row0 : row0 + P, :], in_=out_sb[:])

    _split_multi_waits(nc)
    return nc


_cached_nc = None


def _get_nc():
    global _cached_nc
    if _cached_nc is None:
        _cached_nc = build_nc()
    return _cached_nc


def kernel(x, weight, bias, _want_trace=False, _trace_kwargs=None):
    from concourse.bass_utils import run_bass_kernel_spmd

    x = np.ascontiguousarray(np.asarray(x, dtype=np.float32))
    weight = np.ascontiguousarray(np.asarray(weight, dtype=np.float32))
    bias = np.asarray(bias, dtype=np.float32)

    nc = _get_nc()

    in_maps = []
    for c in range(N_CORES):
        r, h = divmod(c, C)
        xs = np.ascontiguousarray(x[r * TOK_SH : (r + 1) * TOK_SH])
        ws = np.ascontiguousarray(weight[h * OUT_SH : (h + 1) * OUT_SH])
        bs = np.ascontiguousarray(
            np.broadcast_to(bias[h * OUT_SH : (h + 1) * OUT_SH][None, :], (P, OUT_SH))
        )
        in_maps.append({"x": xs, "w": ws, "b": bs})

    res = run_bass_kernel_spmd(
        nc,
        in_maps,
        list(range(N_CORES)),
        trace=_want_trace,
        **(_trace_kwargs or {}),
    )

    out = np.empty((N_TOK, OUT_F), dtype=np.float32)
    for c in range(N_CORES):
        r, h = divmod(c, C)
        out[r * TOK_SH : (r + 1) * TOK_SH, h * OUT_SH : (h + 1) * OUT_SH] = res.results[
            c
        ]["y"]

    if _want_trace:
        kernel.last_results = res
    return out


def _make_sharded_fn(nc, n_cores=N_CORES):
    """Replicate bass2jax.run_bass_via_pjrt's jit so executions can be timed
    with device-resident inputs (the stock entry point re-transfers inputs and
    returns numpy, hiding the kernel time)."""
    import jax
    from jax.sharding import Mesh, PartitionSpec
    from jax.experimental.shard_map import shard_map

    from concourse import bass2jax
    from concourse import mybir as mb

    bass2jax.install_neuronx_cc_hook()

    partition_name = nc.partition_id_tensor.name if nc.partition_id_tensor else None
    in_names, out_names, out_avals, zero_outs = [], [], [], []
    for alloc in nc.m.functions[0].allocations:
        if not isinstance(alloc, mb.MemoryLocationSet):
            continue
        name = alloc.memorylocations[0].name
        if alloc.kind == "ExternalInput":
            if name != partition_name:
                in_names.append(name)
        elif alloc.kind == "ExternalOutput":
            out_names.append(name)
            shape = tuple(alloc.tensor_shape)
            dtype = mb.dt.np(alloc.dtype)
            out_avals.append(jax.core.ShapedArray(shape, dtype))
            zero_outs.append(np.zeros(shape, dtype))
    n_params = len(in_names)
    n_outs = len(out_avals)
    in_names = in_names + out_names
    if partition_name is not None:
        in_names.append(partition_name)
    donate = tuple(range(n_params, n_params + n_outs))

    def _body(*args):
        operands = list(args)
        if partition_name is not None:
            operands.append(bass2jax.partition_id_tensor())
        outs = bass2jax._bass_exec_p.bind(
            *operands,
            out_avals=tuple(out_avals),
            in_names=tuple(in_names),
            out_names=tuple(out_names),
            lowering_input_output_aliases=(),
            sim_require_finite=True,
            sim_require_nnan=True,
            nc=nc,
        )
        return tuple(outs)

    devices = jax.devices()[:n_cores]
    mesh = Mesh(np.asarray(devices), ("core",))
    in_specs = (PartitionSpec("core"),) * (n_params + n_outs)
    out_specs = (PartitionSpec("core"),) * len(out_names)
    sharded = jax.jit(
        shard_map(
            _body, mesh=mesh, in_specs=in_specs, out_specs=out_specs, check_rep=False
        ),
        donate_argnums=donate,
        keep_unused=True,
    )
    return sharded, in_names[:n_params], zero_outs


def _time_sharded(nc, in_maps, iters):
    import jax

    sharded, in_names, zero_outs = _make_sharded_fn(nc)
    concat_in = [
        np.concatenate([np.asarray(m[name]) for m in in_maps], axis=0)
        for name in in_names
    ]
    dev_in = [jax.device_put(a) for a in concat_in]
    times = []
    out = None
    for _ in range(iters + 1):
        zeros = [
            jax.device_put(np.zeros((N_CORES * z.shape[0], *z.shape[1:]), z.dtype))
            for z in zero_outs
        ]
        for z in zeros:
            z.block_until_ready()
        t0 = time.perf_counter()
        out = sharded(*dev_in, *zeros)
        for o in out:
            o.block_until_ready()
        times.append(time.perf_counter() - t0)
    return sorted(times[1:])[len(times[1:]) // 2], out  # median, skip warmup


def _build_null_nc():
    nc = bass.Bass()
    x = nc.declare_dram_parameter("x", [P, 512], f32, isOutput=False)
    y = nc.declare_dram_parameter("y", [P, 512], f32, isOutput=True)
    with tile.TileContext(nc) as tc:
        with tc.tile_pool(name="p", bufs=1) as pool:
            t = pool.tile([P, 512], f32)
            nc.sync.dma_start(out=t[:], in_=x[:])
            nc.sync.dma_start(out=y[:], in_=t[:])
    _split_multi_waits(nc)
    return nc


def time_kernel_ns(inputs, iters=10):
    """Median wall-clock per execution with device-resident inputs, minus the
    dispatch overhead measured on a near-empty kernel. Also reports raw."""
    import time as _t

    x = np.ascontiguousarray(np.asarray(inputs["x"], dtype=np.float32))
    weight = np.ascontiguousarray(np.asarray(inputs["weight"], dtype=np.float32))
    bias = np.asarray(inputs["bias"], dtype=np.float32)
    in_maps = []
    for c in range(N_CORES):
        r, h = divmod(c, C)
        in_maps.append(
            {
                "x": np.ascontiguousarray(x[r * TOK_SH : (r + 1) * TOK_SH]),
                "w": np.ascontiguousarray(weight[h * OUT_SH : (h + 1) * OUT_SH]),
                "b": np.ascontiguousarray(
                    np.broadcast_to(
                        bias[h * OUT_SH : (h + 1) * OUT_SH][None, :], (P, OUT_SH)
                    )
                ),
            }
        )
    t_main, _ = _time_sharded(_get_nc(), in_maps, iters)

    null_nc = _build_null_nc()
    null_maps = [{"x": np.zeros((P, 512), np.float32)} for _ in range(N_CORES)]
    t_null, _ = _time_sharded(null_nc, null_maps, iters)

    print(f"raw median wall: {t_main * 1e9:.0f} ns; null-kernel wall: {t_null * 1e9:.0f} ns")
    return (t_main - t_null) * 1e9


# revision 8
# speedup vs baseline: 1.1883x; 1.1883x over previous
"""BinaryLinear v4: all transposes on the PE, no DRAM staging.

Per core (grid R=4 token-groups x C=2 col-groups):
  x/w tiles: DMA f32 -> ACT sign -> bf16 -> PE transpose (128x128, batched 4
  per PSUM tile) -> DVE copy to fp8 (+-1) subtile-layout tiles.
  Matmul: fp8 DoubleRow over k-subtile pairs, PSUM f32, bias on eviction.
HBM traffic per core is just x/4 + w/2 + out/8 + bias: ~68 MB.
"""

import os
import sys
import time

sys.path.insert(0, "/opt/trn_rl_repo")

if "jax" not in sys.modules and os.environ.get("JAX_PLATFORMS") in ("cpu",):
    del os.environ["JAX_PLATFORMS"]

import numpy as np

import concourse.bass as bass
import concourse.mybir as mybir
import concourse.tile as tile
from concourse.masks import make_identity

N_TOK = 8192
IN_F = 4096
OUT_F = 4096
R = 4
C = 2
N_CORES = 8
TOK_SH = N_TOK // R  # 2048
OUT_SH = OUT_F // C  # 2048
P = 128
KS = IN_F // P  # 32
NQ = 4
TOKQ = TOK_SH // NQ  # 512
O_CHUNK = 512
N_OCH = OUT_SH // O_CHUNK  # 4
KG = 4  # transposes batched per PSUM staging tile

f32 = mybir.dt.float32
bf16 = mybir.dt.bfloat16
fp8 = mybir.dt.float8e4


def _split_multi_waits(nc, limit=1):
    """walrus here allows one sync-wait per instruction; move extras onto
    preceding NoOps (engines are in-order, so semantics are unchanged)."""
    for f in nc.m.functions:
        for bb in f.blocks:
            new = []
            for inst in bb.instructions:
                si = inst.sync_info
                if si is not None and len(si.on_wait) > limit:
                    waits = list(si.on_wait)
                    extra, keep = waits[:-limit], waits[-limit:]
                    for j, w in enumerate(extra):
                        new.append(
                            mybir.InstNoOp(
                                name=f"{inst.name}-w{j}",
                                engine=inst.engine,
                                sync_info=mybir.SyncInfo(on_wait=[w], on_update=[]),
                            )
                        )
                    inst.sync_info = mybir.SyncInfo(
                        on_wait=keep, on_update=list(si.on_update)
                    )
                new.append(inst)
            bb.instructions = new


def build_nc(repeat=1):
    nc = bass.Bass()
    x = nc.declare_dram_parameter("x", [TOK_SH, IN_F], f32, isOutput=False)
    w = nc.declare_dram_parameter("w", [OUT_SH, IN_F], f32, isOutput=False)
    b = nc.declare_dram_parameter("b", [P, OUT_SH], f32, isOutput=False)
    y = nc.declare_dram_parameter("y", [TOK_SH, OUT_SH], f32, isOutput=True)

    HALF = IN_F // 2  # column half processed per load tile
    KSH = KS // 2  # k-subtiles per half (16)

    with tile.TileContext(nc) as tc:
        with (
            tc.tile_pool(name="const", bufs=1) as const,
            tc.tile_pool(name="a_in", bufs=3) as a_in,
            tc.tile_pool(name="a_bf", bufs=3) as a_bf,
            tc.tile_pool(name="wbt", bufs=1) as wbt_pool,
            tc.tile_pool(name="xbt", bufs=2) as xbt_pool,
            tc.tile_pool(name="psum", bufs=4, space="PSUM") as psum_pool,
            tc.tile_pool(name="psum_t", bufs=3, space="PSUM") as psum_t_pool,
            tc.tile_pool(name="outp", bufs=4) as out_pool,
        ):
            bias_bc = const.tile([P, OUT_SH], f32)
            nc.sync.dma_start(out=bias_bc[:], in_=b[:])
            ident = const.tile([P, P], bf16)
            make_identity(nc, ident)

            wbt = wbt_pool.tile([P, KS, OUT_SH], fp8)

            def sign_transpose_tile(src, row_tile, dst, dst_free0):
                """Load [128, IN_F] rows row_tile of src, binarize, transpose
                into dst[:, :, dst_free0:dst_free0+128] (fp8, +-1)."""
                for kh in range(2):
                    a = a_in.tile([P, HALF], f32, tag="a_in")
                    nc.sync.dma_start(
                        out=a[:],
                        in_=src[
                            row_tile * P : (row_tile + 1) * P,
                            kh * HALF : (kh + 1) * HALF,
                        ],
                    )
                    s = a_bf.tile([P, HALF], bf16, tag="a_bf")
                    nc.scalar.sign(s[:], a[:])
                    for kg in range(KSH // KG):
                        pst = psum_t_pool.tile([P, KG * P], bf16, tag="pst")
                        for j in range(KG):
                            nc.tensor.transpose(
                                pst[:, j * P : (j + 1) * P],
                                s[:, (kg * KG + j) * P : (kg * KG + j + 1) * P],
                                ident,
                            )
                        ks0 = kh * KSH + kg * KG
                        nc.vector.tensor_copy(
                            dst[:, ks0 : ks0 + KG, dst_free0 : dst_free0 + P],
                            pst[:].rearrange("p (g c) -> p g c", g=KG),
                        )

            def emit_w_tile(ot):
                sign_transpose_tile(w, ot, wbt, ot * P)

            def emit_x_quarter(q, xbt):
                for rt in range(TOKQ // P):
                    sign_transpose_tile(x, q * (TOKQ // P) + rt, xbt, rt * P)

            def emit_mm(q, oc, xbt):
                for nt in range(TOKQ // P):
                    ps = psum_pool.tile([P, O_CHUNK], f32, tag="ps")
                    for kp in range(KS // 2):
                        nc.tensor.matmul(
                            ps[:],
                            lhsT=xbt[:, 2 * kp : 2 * kp + 2, nt * P : (nt + 1) * P],
                            rhs=wbt[
                                :, 2 * kp : 2 * kp + 2, oc * O_CHUNK : (oc + 1) * O_CHUNK
                            ],
                            start=(kp == 0),
                            stop=(kp == KS // 2 - 1),
                            perf_mode=mybir.MatmulPerfMode.DoubleRow,
                        )
                    out_sb = out_pool.tile([P, O_CHUNK], f32, tag="out_sb")
                    nc.vector.tensor_add(
                        out=out_sb[:],
                        in0=ps[:],
                        in1=bias_bc[:, oc * O_CHUNK : (oc + 1) * O_CHUNK],
                    )
                    row0 = q * TOKQ + nt * P
                    nc.sync.dma_start(
                        out=y[row0 : row0 + P, oc * O_CHUNK : (oc + 1) * O_CHUNK],
                        in_=out_sb[:],
                    )

            # Emission order pipelines w column-groups against the q0 matmuls:
            # MMs for o-chunk oc only need w row-tiles 4*oc..4*oc+3.
            def body():
                xbt0 = xbt_pool.tile([P, KS, TOKQ], fp8, tag="xbt")
                emit_x_quarter(0, xbt0)
                for oc in range(N_OCH):
                    for ot in range(4 * oc, 4 * oc + 4):
                        emit_w_tile(ot)
                    emit_mm(0, oc, xbt0)
                for q in range(1, NQ):
                    xbt = xbt_pool.tile([P, KS, TOKQ], fp8, tag="xbt")
                    emit_x_quarter(q, xbt)
                    for oc in range(N_OCH):
                        emit_mm(q, oc, xbt)

            if repeat == 1:
                body()
            else:
                with tc.For_i(0, repeat, 1):
                    body()

    _split_multi_waits(nc)
    return nc


_cached_nc = None


def _get_nc():
    global _cached_nc
    if _cached_nc is None:
        _cached_nc = build_nc()
    return _cached_nc


def _in_maps(x, weight, bias):
    maps = []
    for c in range(N_CORES):
        r, h = divmod(c, C)
        maps.append(
            {
                "x": np.ascontiguousarray(x[r * TOK_SH : (r + 1) * TOK_SH]),
                "w": np.ascontiguousarray(weight[h * OUT_SH : (h + 1) * OUT_SH]),
                "b": np.ascontiguousarray(
                    np.broadcast_to(
                        bias[h * OUT_SH : (h + 1) * OUT_SH][None, :], (P, OUT_SH)
                    )
                ),
            }
        )
    return maps


def kernel(x, weight, bias):
    from concourse.bass_utils import run_bass_kernel_spmd

    x = np.ascontiguousarray(np.asarray(x, dtype=np.float32))
    weight = np.ascontiguousarray(np.asarray(weight, dtype=np.float32))
    bias = np.asarray(bias, dtype=np.float32)

    res = run_bass_kernel_spmd(_get_nc(), _in_maps(x, weight, bias), list(range(N_CORES)))

    out = np.empty((N_TOK, OUT_F), dtype=np.float32)
    for c in range(N_CORES):
        r, h = divmod(c, C)
        out[r * TOK_SH : (r + 1) * TOK_SH, h * OUT_SH : (h + 1) * OUT_SH] = res.results[
            c
        ]["y"]
    return out


def time_kernel_ns(inputs, k1=2, k2=42, reps=5):
    """HW time per kernel execution, measured as the slope between two
    hardware-loop variants (repeat=k1 vs repeat=k2) so the multi-ms axon
    dispatch cost cancels exactly."""
    import jax
    from jax.sharding import Mesh, PartitionSpec
    from jax.experimental.shard_map import shard_map
    from concourse import bass2jax
    from concourse import mybir as mb

    x = np.ascontiguousarray(np.asarray(inputs["x"], dtype=np.float32))
    weight = np.ascontiguousarray(np.asarray(inputs["weight"], dtype=np.float32))
    bias = np.asarray(inputs["bias"], dtype=np.float32)
    in_maps = _in_maps(x, weight, bias)

    def make_fn(nc):
        bass2jax.install_neuronx_cc_hook()
        partition_name = nc.partition_id_tensor.name if nc.partition_id_tensor else None
        in_names, out_names, out_avals, zero_outs = [], [], [], []
        for alloc in nc.m.functions[0].allocations:
            if not isinstance(alloc, mb.MemoryLocationSet):
                continue
            name = alloc.memorylocations[0].name
            if alloc.kind == "ExternalInput":
                if name != partition_name:
                    in_names.append(name)
            elif alloc.kind == "ExternalOutput":
                out_names.append(name)
                shape = tuple(alloc.tensor_shape)
                dtype = mb.dt.np(alloc.dtype)
                out_avals.append(jax.core.ShapedArray(shape, dtype))
                zero_outs.append(np.zeros(shape, dtype))
        n_params = len(in_names)
        all_in = in_names + out_names
        if partition_name is not None:
            all_in.append(partition_name)

        def _body(*args):
            operands = list(args)
            if partition_name is not None:
                operands.append(bass2jax.partition_id_tensor())
            return tuple(
                bass2jax._bass_exec_p.bind(
                    *operands,
                    out_avals=tuple(out_avals),
                    in_names=tuple(all_in),
                    out_names=tuple(out_names),
                    lowering_input_output_aliases=(),
                    sim_require_finite=True,
                    sim_require_nnan=True,
                    nc=nc,
                )
            )

        devices = jax.devices()[:N_CORES]
        mesh = Mesh(np.asarray(devices), ("core",))
        nin = n_params + len(out_names)
        fn = jax.jit(
            shard_map(_body, mesh=mesh, in_specs=(PartitionSpec("core"),) * nin,
                      out_specs=(PartitionSpec("core"),) * len(out_names), check_rep=False),
            keep_unused=True,
        )
        return fn, in_names[:n_params], zero_outs

    def measure(nc):
        fn, names, zero_outs = make_fn(nc)
        dev_in = [
            jax.device_put(np.concatenate([np.asarray(m[nm]) for m in in_maps], axis=0))
            for nm in names
        ]
        dev_zero = [
            jax.device_put(np.zeros((N_CORES * z.shape[0], *z.shape[1:]), z.dtype))
            for z in zero_outs
        ]
        for a in dev_in + dev_zero:
            a.block_until_ready()
        out = fn(*dev_in, *dev_zero)
        for o in out:
            o.block_until_ready()
        ts = []
        for _ in range(reps):
            t0 = time.perf_counter()
            out = fn(*dev_in, *dev_zero)
            for o in out:
                o.block_until_ready()
            ts.append(time.perf_counter() - t0)
        ts.sort()
        return ts[len(ts) // 2]

    t1 = measure(build_nc(repeat=k1))
    t2 = measure(build_nc(repeat=k2))
    return (t2 - t1) / (k2 - k1) * 1e9
